# revision 58
# baseline (speedup 1.0000x reference)
"""Trainium2 Bass kernel for nn_CNN2DAttWind_NL (conv + 32-head rank-1 attention + MLP).

Contract: kernel(**inputs) takes FULL unsharded numpy inputs, returns FULL output
[8192, 5] float32.  Internally shards the batch across 8 NeuronCores (pure data
parallel) and runs a Bass/Tile kernel via run_bass_kernel_spmd.

Layout strategy per core (B_local = 1024):
  - conv as K=32 im2col matmuls over dx-shifted contiguous windows (dy is
    a 2-step PSUM accumulation), M=128 out-channels (32 conv_out + 32 q +
    32 k + 32 v), cols=(b, 25 output pixels).
  - attention: partitions = (b mod 4, head) = 128 rows; free = (b', j, i)
    with i innermost.  z = q_i*k_j on DVE (2x fp16 mode; k and v carry the
    pair-duplication trick, q is the step-1 operand), exp on ACT, Ev = E*v
    on DVE.  The 25-wide softmax num/den segment sums run on the
    TensorEngine as 25 accumulating identity-weight matmuls into fp32
    PSUM: with i innermost each j-slice is contiguous 50B runs, so the PE
    streams at full rate; E and Ev share one [128,2,...] tile so each
    j-slice matmul reduces both at once.  One explicit identity ldweights
    per group; the matmuls skip their per-matmul self-load.
  - 1x1 conv contracts v-channels inside the b4 partition blocks of
    pair-major attn_t (block-diag aw, 2 blocks/matmul at bases 0/64) -> no
    attention re-shuffle DMAs.  qkv shuffles ride GpSimd SWDGE, keeping
    the sync queue free.
  - 2-deep software pipeline: conv/evict/shuffle for chunk c+2 is emitted
    inside chunk c's window and z/exp for chunk c+1's first subs are
    prefetched at chunk c's tail, so neither the ACT eviction backlog nor
    the shuffle chain sits in front of the exp -> Ev -> reduction critical
    path.
  - 3-layer MLP as PE matmuls (dense1 accumulates over the 25 pixels in
    PSUM; biases applied via ACT bias on eviction).
"""

import os
from contextlib import ExitStack

import numpy as np

# Problem constants (hardcoded per harness contract)
B, CIN, HIN, WIN = 8192, 16, 6, 6
FM = 64
DK = FM // 2          # 32
DV = FM // 2          # 32
NH = FM // 2          # 32 heads, dkh = dvh = 1
OUT_CH = 5
HID = 128
HW = 25               # 5x5 output pixels

NCORES = 8
BL = B // NCORES      # 1024 samples per core
CB = 128              # samples per outer chunk
NCHUNK = BL // CB     # 8
BP = CB // 4          # 32 b' groups per chunk (pair partition = (b%4, head))
SB = 8                # b' groups per attention sub-chunk -> 32 samples
NSUB = BP // SB       # 4


def _build_program(debug_dump=False, repeat=1):
    """Build the Bass program once; returns (nc, input_names, output_name)."""
    import concourse.bass as bass
    import concourse.tile as tile
    from concourse import bacc, mybir

    f32 = mybir.dt.float32
    f16 = mybir.dt.float16
    AF = mybir.ActivationFunctionType
    ALU = mybir.AluOpType
    AX = mybir.AxisListType

    nc = bacc.Bacc(
        "TRN2",
        target_bir_lowering=False,
        debug=False,
        enable_asserts=False,
        num_devices=NCORES,
    )

    # ---- DRAM I/O ----
    x_d = nc.dram_tensor("x_s", [BL, CIN, HIN, WIN], f16, kind="ExternalInput")
    wp32_d = nc.dram_tensor("wp32", [128, 5], f32, kind="ExternalInput")
    wp16_d = nc.dram_tensor("wp16", [128, 4229], f16, kind="ExternalInput")
    out_d = nc.dram_tensor("out", [5, BL], f32, kind="ExternalOutput")
    dbg = {}
    if debug_dump:
        dbg["qkv"] = nc.dram_tensor("dbg_qkv", [96, CB, 25], f16, kind="ExternalOutput")
        dbg["att"] = nc.dram_tensor("dbg_att", [128, BP, 25], f16, kind="ExternalOutput")
        dbg["yt"] = nc.dram_tensor("dbg_yt", [64, CB, 25], f16, kind="ExternalOutput")
        dbg["y1"] = nc.dram_tensor("dbg_y1", [128, CB], f16, kind="ExternalOutput")

    in_names = [t.name for t in (x_d, wp32_d, wp16_d)]

    with tile.TileContext(nc) as tc, ExitStack() as ctx:
        singles = ctx.enter_context(tc.tile_pool(name="singles", bufs=1))
        imc_p = ctx.enter_context(tc.tile_pool(name="imc", bufs=2))
        qkv_p = ctx.enter_context(tc.tile_pool(name="qkv", bufs=2))
        yt_p = ctx.enter_context(tc.tile_pool(name="yt", bufs=3))
        qt_p = ctx.enter_context(tc.tile_pool(name="qt", bufs=3))
        att_p = ctx.enter_context(tc.tile_pool(name="att", bufs=2))
        z_p = ctx.enter_context(tc.tile_pool(name="z", bufs=2))
        e_p = ctx.enter_context(tc.tile_pool(name="e", bufs=4))
        red_p = ctx.enter_context(tc.tile_pool(name="red", bufs=2))
        mlp_p = ctx.enter_context(tc.tile_pool(name="mlp", bufs=1))
        outp_p = ctx.enter_context(tc.tile_pool(name="outp", bufs=2))
        ps_conv = ctx.enter_context(tc.tile_pool(name="psc", bufs=2, space="PSUM"))
        ps_att = ctx.enter_context(tc.tile_pool(name="psa", bufs=1, space="PSUM"))
        ps_mlp = ctx.enter_context(tc.tile_pool(name="psm", bufs=1, space="PSUM"))
        ps_red = ctx.enter_context(tc.tile_pool(name="psr", bufs=2, space="PSUM"))

        # ---- weight loads: small conv/attn weights first so chunk-0's x
        # load isn't stuck behind the 835KB MLP-weight transfer (that part
        # is issued inside the c==0 body, after the first x DMA) ----
        wp32 = singles.tile([128, 5], f32)
        wp16 = singles.tile([128, 4229], f16)
        # wconv (16KB) gates the first conv matmul: load it before anything
        # else; conv biases next; aw/w3/identity later; the 835KB MLP block
        # last (inside the c==0 body, behind the first x DMA).
        nc.sync.dma_start(out=wp16[0:32, 3973:4229],
                          in_=wp16_d.ap()[0:32, 3973:4229])
        nc.sync.dma_start(out=wp32, in_=wp32_d.ap())
        nc.sync.dma_start(out=wp16[:, 3264:3333],
                          in_=wp16_d.ap()[:, 3264:3333])
        nc.sync.dma_start(out=wp16[:, 3845:3973],
                          in_=wp16_d.ap()[:, 3845:3973])
        cb_s = wp32[:, 0:1]
        expb_s = wp32[:, 1:2]
        b1_s = wp32[:, 2:3]
        b2_s = wp32[0:64, 3:4]
        ab_s = wp32[:, 4:5]             # attn bias tiled 4x across partitions
        w2_s = wp16[:, 0:64]
        w1_s = wp16[0:64, 64:3264].rearrange("p (h i) -> p h i", i=25)
        aw_s = wp16[:, 3264:3328]       # block-diag aw per (b4-pair, b4%2)
        w3_s = wp16[0:64, 3328:3333]
        ident_s = wp16[:, 3845:3973]    # fp16 eye(128) for j-sum matmuls
        # K=32 im2col conv weights, one [32,128] set per dy
        wconv32_s = wp16[0:32, 3973:4229].rearrange("p (d c) -> p d c", d=2)

        NB = 512 // 25                              # 20 samples per col-tile
        NTILE = (CB + NB - 1) // NB                 # 7 col-tiles per chunk

        def conv_head(c):
            b0 = (c % NCHUNK) * CB
            # ---- x load as K=32 im2col over dx: xa32[(dx, cin), b, 35] ----
            # two dx-shifted contiguous-window DMAs (3-dim APs -- the DMA
            # layer caps at 3 dims); dy becomes a 2-step PSUM accumulation.
            xa32 = imc_p.tile([32, CB, 36], f16)
            for dx in range(2):
                src = bass.AP(tensor=x_d, offset=b0 * 576 + dx,
                              ap=[[36, 16], [576, CB], [1, 35]])
                nc.sync.dma_start(
                    out=xa32[16 * dx:16 * dx + 16, :, 0:35], in_=src)
            if c == 0:
                nc.sync.dma_start(out=wp16[:, 0:3264],
                                  in_=wp16_d.ap()[:, 0:3264])
            qkv_s = qkv_p.tile([96, CB, 25], f16)   # rows: q(0:32) k(32:64) v(64:96)
            y_t = yt_p.tile([64, CB, 25], f16)      # 0:32 relu(conv_out), 32:64 attn
            q_t = qt_p.tile([128, BP, 26], f16, tag="q_t")     # lane 25 pad
            k_t = qt_p.tile([128, BP, 25], f16, tag="k_t")
            k_dd = qt_p.tile([128, BP, 25, 2], f16, tag="k_dd")
            v_t = qt_p.tile([128, BP, 25], f16, tag="v_t")
            v_dd = qt_p.tile([128, BP, 25, 2], f16, tag="v_dd")
            return dict(xa32=xa32, qkv_s=qkv_s, y_t=y_t, q_t=q_t, k_t=k_t,
                        k_dd=k_dd, v_t=v_t, v_dd=v_dd)

        def conv_tiles(st, t_lo, t_hi):
            # ---- conv as 2 accumulating K=32 matmuls per col-tile;
            # emitted in pieces interleaved between reduction groups so
            # the PE/ACT streams never see a long conv-only block ----
            xa32, qkv_s, y_t = st["xa32"], st["qkv_s"], st["y_t"]
            for t in range(t_lo, min(t_hi, NTILE)):
                s0 = t * NB
                sb_n = min(NB, CB - s0)
                tn = sb_n * 25
                pscv = ps_conv.tile([128, 512], f32)
                for dy in range(2):
                    rhs = bass.AP(
                        tensor=xa32.tensor,
                        offset=xa32.offset + s0 * 36 + dy * 6,
                        ap=[xa32.ap[0], [36, sb_n], [6, 5], [1, 5]],
                    )
                    nc.tensor.matmul(pscv[:, :tn], wconv32_s[:, dy],
                                     rhs, start=(dy == 0), stop=(dy == 1))
                # qkv rows (0:96) first: the shuffle chain waits on these
                nc.scalar.activation(
                    qkv_s[:, s0:s0 + sb_n],
                    pscv[0:96, :tn].rearrange("p (b i) -> p b i", i=25),
                    AF.Identity, bias=cb_s[0:96], scale=1.0)
                nc.scalar.activation(
                    y_t[0:32, s0:s0 + sb_n],
                    pscv[96:128, :tn].rearrange("p (b i) -> p b i", i=25),
                    AF.Relu, bias=cb_s[96:128], scale=1.0)

        def conv_shuffles(st):
            # ---- shuffle qkv -> pair-major layout [ (b4, n), b', . ] ----
            # z/E/Ev free dims are (b', j, i) with i innermost, so the
            # per-j reduction matmul rhs slices are contiguous 50B runs.
            qkv_v = st["qkv_s"].rearrange("p (b2 f) i -> p f b2 i", f=4)
            # issue order q -> k -> v: SWDGE issue is ~1us serial per DMA,
            # and the z chain needs q_t + k_dd first, v_dd last
            for b4 in range(4):
                nc.gpsimd.dma_start(out=st["q_t"][32 * b4:32 * b4 + 32, :, 0:25],
                                    in_=qkv_v[0:32, b4])
            for b4 in range(4):
                nc.gpsimd.dma_start(out=st["k_t"][32 * b4:32 * b4 + 32],
                                    in_=qkv_v[32:64, b4])
            for b4 in range(4):
                nc.gpsimd.dma_start(out=st["v_t"][32 * b4:32 * b4 + 32],
                                    in_=qkv_v[64:96, b4])

        def dup_stage(st):
            # pair-duplication copies on DVE, emitted after the z/exp
            # prefetch of the nearer chunk: any stall waiting for the k/v
            # shuffles lands in DVE's boundary idle, never ahead of an exp
            # on the ACT queue.
            nc.vector.tensor_copy(st["k_dd"][:, :, :, 0:1],
                                  st["k_t"].unsqueeze(3))
            nc.vector.tensor_copy(st["k_dd"][:, :, :, 1:2],
                                  st["k_t"].unsqueeze(3))
            nc.vector.tensor_copy(st["v_dd"][:, :, :, 0:1],
                                  st["v_t"].unsqueeze(3))
            nc.vector.tensor_copy(st["v_dd"][:, :, :, 1:2],
                                  st["v_t"].unsqueeze(3))

        def emit_z(st, s):
            q_t, k_dd = st["q_t"], st["k_dd"]
            z = z_p.tile([128, SB, 25, 26], f16)
            q_bc = bass.AP(tensor=q_t.tensor, offset=q_t.offset + SB * s * 26,
                           ap=[q_t.ap[0], [26, SB], [0, 25], [1, 26]])
            k_bc = bass.AP(tensor=k_dd.tensor, offset=k_dd.offset + SB * s * 50,
                           ap=[k_dd.ap[0], [50, SB], [2, 25], [0, 13], [1, 2]])
            nc.vector.tensor_mul(z, q_bc, k_bc)
            return z

        def emit_exp(z):
            # T[:,0] = E = exp(z+bias); T[:,1] = Ev (written by the DVE mul).
            # The i=25 pad lane is never read by the reduction (it streams
            # i 0:25), so no pad memsets are needed anywhere.
            T = e_p.tile([128, 2, SB, 25, 26], f16)
            nc.scalar.activation(T[:, 0, :, :, 0:25], z[:, :, :, 0:25],
                                 AF.Exp, bias=expb_s, scale=1.0)
            return T

        def attn_subs(c, st, pre, stn2=None):
            """Attention subs 0..3 of chunk c.  pre = (z-tiles for subs 0/1,
            T tile with sub-0's exp already emitted) prefetched by the
            previous chunk's tail, or None for chunk 0.  stn2 = chunk c+2's
            conv state: its col-tiles are emitted two per sub (plus the
            shuffles after the last), so conv MMs/evicts spread across the
            window instead of forming a boundary block."""
            v_dd = st["v_dd"]
            attn_t = att_p.tile([128, BP, 25], f16, tag="attn_t")
            st["attn_t"] = attn_t
            if pre is None:
                zq = [emit_z(st, 0), emit_z(st, 1)]
                Ts = {}
            else:
                zq = list(pre["z"])
                Ts = {0: pre["T0"]}
                if "T1" in pre:
                    Ts[1] = pre["T1"]
            for s in range(NSUB):
                sl = slice(SB * s, SB * (s + 1))
                v_bc = bass.AP(tensor=v_dd.tensor, offset=v_dd.offset + SB * s * 50,
                               ap=[v_dd.ap[0], [50, SB], [2, 25], [0, 13], [1, 2]])
                T = Ts.pop(s, None)
                ev_done = (s == 0 and T is not None and pre is not None
                           and pre.get("ev0"))
                if T is None:
                    T = emit_exp(zq[s])
                if s + 2 < NSUB:
                    zq.append(emit_z(st, s + 2))
                if not ev_done:
                    nc.vector.tensor_mul(T[:, 1], T[:, 0], v_bc)
                # j-sum on PE: psr[:,0] = den = sum_j E, psr[:,1] = num.
                # One explicit identity ldweights per group; the 25
                # accumulating matmuls skip their per-matmul self-load.
                psr = ps_red.tile([128, 2, SB, 25], f32)
                psr2 = psr.rearrange("p a b i -> p (a b i)")
                nc.tensor.ldweights(ident_s)
                for j in range(25):
                    mm = nc.tensor.matmul(psr2, ident_s, T[:, :, :, j, 0:25],
                                          start=(j == 0), stop=(j == 24))
                    mm.ins.ldweights = False
                den, num = psr[:, 0], psr[:, 1]
                rden = red_p.tile([128, SB, 25, 1], f32, tag="rden")
                nc.vector.reciprocal_approx_fast(
                    rden.rearrange("p b i o -> p (b i o)"),
                    den.rearrange("p b i -> p (b i)"))
                nc.vector.tensor_mul(attn_t[:, sl, :].unsqueeze(3),
                                     num.unsqueeze(3), rden)
                if stn2 is not None:
                    conv_tiles(stn2, 2 * s, 2 * s + 2)
                    if s == NSUB - 1:
                        conv_tiles(stn2, 2 * NSUB, NTILE)
                        conv_shuffles(stn2)

        def prefetch_next(stn):
            """Cross-chunk prefetch: z for subs 0/1 plus exp AND Ev for sub
            0 of the next chunk.  Emitted at the tail, after all of this
            chunk's DVE ops, so it can't delay them -- the next chunk's
            first reduction group is fully fed when the PE tail drains."""
            zn0 = emit_z(stn, 0)
            zn1 = emit_z(stn, 1)
            T0 = emit_exp(zn0)
            v_bc = bass.AP(tensor=stn["v_dd"].tensor, offset=stn["v_dd"].offset,
                           ap=[stn["v_dd"].ap[0], [50, SB], [2, 25], [0, 13],
                               [1, 2]])
            nc.vector.tensor_mul(T0[:, 1], T0[:, 0], v_bc)
            # exp(next,1) prefetched too (its Ev stays in-loop: prefetching
            # it would push z(next,2) behind an exp wait on the in-order DVE)
            T1 = emit_exp(zn1)
            return {"z": [zn0, zn1], "T0": T0, "T1": T1, "ev0": True}

        def attn_tail(c, st):
            b0 = (c % NCHUNK) * CB
            y_t, attn_t = st["y_t"], st["attn_t"]
            # ---- 1x1 conv + relu -> y_t rows 32:64 ----
            # contract v-channels inside the b4 partition blocks of pair-major
            # attn_t directly (block-diag aw, two b4 per matmul at bases 0/64)
            # -> no attention re-shuffle DMAs.
            for g in range(2):
                atf = attn_t[64 * g:64 * g + 64].rearrange("p b i -> p (b i)")
                psat = ps_att.tile([64, BP * 25], f32)
                for t0 in range(0, BP * 25, 512):
                    tn = min(512, BP * 25 - t0)
                    nc.tensor.matmul(psat[:, t0:t0 + tn],
                                     aw_s[64 * g:64 * g + 64],
                                     atf[:, t0:t0 + tn], start=True, stop=True)
                for h in range(2):
                    b4 = 2 * g + h
                    # y_t columns for samples b = b4 + 4*b'
                    ysl = y_t[32:64]
                    ydst = bass.AP(tensor=ysl.tensor,
                                   offset=ysl.offset + b4 * 25,
                                   ap=[ysl.ap[0], [100, BP], [1, 25]])
                    nc.scalar.activation(
                        ydst,
                        psat[32 * h:32 * h + 32].rearrange(
                            "p (b i) -> p b i", i=25),
                        AF.Relu, bias=ab_s[64 * g + 32 * h:64 * g + 32 * h + 32],
                        scale=1.0)

            # ---- dense1: accumulate over 25 pixels ----
            ps1 = ps_mlp.tile([128, CB], f32, tag="ps1")
            for i in range(25):
                nc.tensor.matmul(ps1, w1_s[:, :, i], y_t[:, :, i],
                                 start=(i == 0), stop=(i == 24))
            y1 = mlp_p.tile([128, CB], f16, tag="y1")
            nc.scalar.activation(y1, ps1, AF.Relu, bias=b1_s, scale=1.0)

            # ---- dense2 (shares one PSUM bank with dense3) ----
            ps23 = ps_mlp.tile([64, 2, CB], f32, tag="ps23")
            nc.tensor.matmul(ps23[:, 0], w2_s, y1, start=True, stop=True)
            y2 = mlp_p.tile([64, CB], f16, tag="y2")
            nc.scalar.activation(y2, ps23[:, 0], AF.Relu, bias=b2_s, scale=1.0)

            if debug_dump and c == 0:
                nc.sync.dma_start(out=dbg["qkv"].ap(), in_=st["qkv_s"])
                nc.sync.dma_start(out=dbg["att"].ap(), in_=attn_t)
                nc.sync.dma_start(out=dbg["yt"].ap(), in_=y_t)
                nc.sync.dma_start(out=dbg["y1"].ap(), in_=y1)

            # ---- dense3 (bias added on host) ----
            nc.tensor.matmul(ps23[0:5, 1], w3_s, y2, start=True, stop=True)
            outs = outp_p.tile([5, CB], f32)
            nc.scalar.copy(outs, ps23[0:5, 1])
            nc.sync.dma_start(
                out=bass.AP(tensor=out_d, offset=b0, ap=[[BL, 5], [1, CB]]),
                in_=outs)

        # 2-deep software pipeline: conv for chunk c+2 is emitted inside
        # chunk c's window (after the attention subs), and z/exp for chunk
        # c+1's first subs are prefetched at chunk c's tail, so neither the
        # ACT eviction backlog nor the shuffle chain ever sits in front of
        # the exp -> Ev -> reduction critical path and the PE never idles
        # long enough for HAM to re-throttle.
        NTOT = NCHUNK * repeat

        def full_conv(c):
            st = conv_head(c)
            conv_tiles(st, 0, NTILE)
            conv_shuffles(st)
            return st

        sts = {0: full_conv(0)}
        dup_stage(sts[0])
        if NTOT > 1:
            sts[1] = full_conv(1)
            dup_stage(sts[1])
        pre = None
        for c in range(NTOT):
            stn2 = None
            if c + 2 < NTOT:
                stn2 = conv_head(c + 2)
                sts[c + 2] = stn2
            attn_subs(c, sts[c], pre, stn2)
            pre = prefetch_next(sts[c + 1]) if c + 1 < NTOT else None
            if stn2 is not None:
                dup_stage(stn2)
            attn_tail(c, sts[c])
            del sts[c]

    nc.finalize()
    return nc, in_names, out_d.name


_PROG = None


def _get_program():
    global _PROG
    if _PROG is None:
        _PROG = _build_program()
    return _PROG


def _host_conv(x, w, b):
    """2x2 VALID conv, NCHW, numpy. Returns [B, O, 25] float32."""
    B_, C_, H_, W_ = x.shape
    out = None
    for dy in range(2):
        for dx in range(2):
            xs = x[:, :, dy:dy + 5, dx:dx + 5].reshape(B_, C_, 25)
            t = np.einsum('oc,bcp->bop', w[:, :, dy, dx], xs,
                          optimize=True)
            out = t if out is None else out + t
    return (out + b[None, :, None]).astype(np.float32)


def _make_in_maps(inputs):
    return _host_prep(**inputs)


def _host_prep(x, conv_w, conv_b, qkv_w, qkv_b, attn_w, attn_b,
               w1, b1, w2, b2, w3, b3):
    # host-side weight prep
    # channel order [q|k|v|conv_out]
    wc = np.concatenate([np.asarray(qkv_w), np.asarray(conv_w)], axis=0)  # [128,16,2,2]
    # wconv32[(dx, cin), dy, ch] for the 2-step K=32 im2col conv matmuls
    wconv32 = np.ascontiguousarray(
        wc.transpose(3, 1, 2, 0).reshape(32, 2, 128)).astype(np.float16)
    cbias = np.concatenate([np.asarray(qkv_b), np.asarray(conv_b)])[:, None].astype(np.float32)
    aw = np.ascontiguousarray(np.asarray(attn_w)[:, :, 0, 0].T).astype(np.float16)
    ab = np.asarray(attn_b)[:, None].astype(np.float32)
    w1t = np.ascontiguousarray(
        np.asarray(w1).reshape(HID, 64, 25).transpose(1, 0, 2)).astype(np.float16)
    b1c = np.asarray(b1)[:, None].astype(np.float32)
    w2t = np.ascontiguousarray(np.asarray(w2).T).astype(np.float16)
    b2c = np.asarray(b2)[:, None].astype(np.float32)
    w3t = np.ascontiguousarray(np.asarray(w3).T).astype(np.float16)

    x = np.asarray(x, dtype=np.float32)

    # exp-overflow guard: softmax is invariant to exp(z - C0); pick C0 from the
    # exact global max of q_i*k_j (corner products of per-row min/max).
    qw, kw = np.asarray(qkv_w)[0:32], np.asarray(qkv_w)[32:64]
    qb_, kb_ = np.asarray(qkv_b)[0:32], np.asarray(qkv_b)[32:64]
    qv = _host_conv(x, qw, qb_)        # [B, 32, 25]
    kv = _host_conv(x, kw, kb_)
    qmax, qmin = qv.max(2), qv.min(2)  # [B, 32]
    kmax, kmin = kv.max(2), kv.min(2)
    zmax = max((qmax * kmax).max(), (qmax * kmin).max(),
               (qmin * kmax).max(), (qmin * kmin).max())
    # margin 5.5: keeps E = exp(z-c0) and Ev comfortably inside fp16 range
    # (num/den partial sums now accumulate in fp32 PSUM).
    c0 = float(max(0.0, zmax - 5.5))
    expb = np.full((128, 1), -c0, dtype=np.float32)

    wp32 = np.zeros((128, 5), np.float32)
    wp32[:, 0:1] = cbias
    wp32[:, 1:2] = expb
    wp32[:, 2:3] = b1c
    wp32[0:64, 3:4] = b2c
    wp32[:, 4:5] = np.tile(ab, (4, 1))
    wp16 = np.zeros((128, 4229), np.float16)
    wp16[:, 0:64] = w2t
    wp16[0:64, 64:3264] = w1t.reshape(64, 3200)
    # block-diag aw: row-block b4 gets aw at col-block (b4 % 2)
    for b4 in range(4):
        wp16[32 * b4:32 * b4 + 32,
             3264 + 32 * (b4 % 2):3296 + 32 * (b4 % 2)] = aw
    wp16[0:64, 3328:3333] = w3t
    wp16[:, 3845:3973] = np.eye(128, dtype=np.float16)
    wp16[0:32, 3973:4229] = wconv32.reshape(32, 256)

    x16 = x.astype(np.float16)
    shared = {"wp32": wp32, "wp16": wp16}
    in_maps = []
    for c in range(NCORES):
        m = dict(shared)
        m["x_s"] = np.ascontiguousarray(x16[c * BL:(c + 1) * BL])
        in_maps.append(m)
    return in_maps


def kernel(x, conv_w, conv_b, qkv_w, qkv_b, attn_w, attn_b,
           w1, b1, w2, b2, w3, b3):
    from concourse.bass_utils import run_bass_kernel_spmd

    nc, in_names, out_name = _get_program()
    in_maps = _host_prep(x, conv_w, conv_b, qkv_w, qkv_b, attn_w, attn_b,
                         w1, b1, w2, b2, w3, b3)
    res = run_bass_kernel_spmd(nc, in_maps, core_ids=list(range(NCORES)))
    outs = [r[out_name] for r in res.results]           # each [5, BL]
    full = np.concatenate([o.T for o in outs], axis=0)  # [8192, 5]
    full = full + np.asarray(b3)[None, :].astype(np.float32)
    return full.astype(np.float32)



# revision 59
# speedup vs baseline: 1.1980x; 1.1980x over previous
"""Trainium2 Bass kernel for nn_CNN2DAttWind_NL (conv + 32-head rank-1 attention + MLP).

Contract: kernel(**inputs) takes FULL unsharded numpy inputs, returns FULL output
[8192, 5] float32.  Internally shards the batch across 8 NeuronCores (pure data
parallel) and runs a Bass/Tile kernel via run_bass_kernel_spmd.

Layout strategy per core (B_local = 1024):
  - conv as K=32 im2col matmuls over dx-shifted contiguous windows (dy is
    a 2-step PSUM accumulation), M=128 out-channels (32 conv_out + 32 q +
    32 k + 32 v), cols=(b, 25 output pixels).
  - attention: partitions = (b mod 4, head) = 128 rows; free = (b', j, i)
    with i innermost.  z = q_i*k_j on DVE (2x fp16 mode; k and v carry the
    pair-duplication trick, q is the step-1 operand), exp on ACT, Ev = E*v
    on DVE.  The 25-wide softmax num/den segment sums run on the
    TensorEngine as 25 accumulating identity-weight matmuls into fp32
    PSUM: with i innermost each j-slice is contiguous 50B runs, so the PE
    streams at full rate; E and Ev share one [128,2,...] tile so each
    j-slice matmul reduces both at once.  One explicit identity ldweights
    per group; the matmuls skip their per-matmul self-load.
  - 1x1 conv contracts v-channels inside the b4 partition blocks of
    pair-major attn_t (block-diag aw, 2 blocks/matmul at bases 0/64) -> no
    attention re-shuffle DMAs.  qkv shuffles ride GpSimd SWDGE, keeping
    the sync queue free.
  - 2-deep software pipeline: conv/evict/shuffle for chunk c+2 is emitted
    inside chunk c's window and z/exp for chunk c+1's first subs are
    prefetched at chunk c's tail, so neither the ACT eviction backlog nor
    the shuffle chain sits in front of the exp -> Ev -> reduction critical
    path.
  - 3-layer MLP as PE matmuls (dense1 accumulates over the 25 pixels in
    PSUM; biases applied via ACT bias on eviction).
"""

import os
from contextlib import ExitStack

import numpy as np

# Problem constants (hardcoded per harness contract)
B, CIN, HIN, WIN = 8192, 16, 6, 6
FM = 64
DK = FM // 2          # 32
DV = FM // 2          # 32
NH = FM // 2          # 32 heads, dkh = dvh = 1
OUT_CH = 5
HID = 128
HW = 25               # 5x5 output pixels

NCORES = 8
BL = B // NCORES      # 1024 samples per core
CB = 128              # samples per outer chunk
NCHUNK = BL // CB     # 8
BP = CB // 4          # 32 b' groups per chunk (pair partition = (b%4, head))
SB = 8                # b' groups per attention sub-chunk -> 32 samples
NSUB = BP // SB       # 4


def _build_program(debug_dump=False, repeat=1):
    """Build the Bass program once; returns (nc, input_names, output_name)."""
    import concourse.bass as bass
    import concourse.tile as tile
    from concourse import bacc, mybir

    f32 = mybir.dt.float32
    f16 = mybir.dt.float16
    AF = mybir.ActivationFunctionType
    ALU = mybir.AluOpType
    AX = mybir.AxisListType

    nc = bacc.Bacc(
        "TRN2",
        target_bir_lowering=False,
        debug=False,
        enable_asserts=False,
        num_devices=NCORES,
    )

    # ---- DRAM I/O ----
    x_d = nc.dram_tensor("x_s", [BL, CIN, HIN, WIN], f16, kind="ExternalInput")
    wp32_d = nc.dram_tensor("wp32", [128, 5], f32, kind="ExternalInput")
    wp16_d = nc.dram_tensor("wp16", [128, 4229], f16, kind="ExternalInput")
    out_d = nc.dram_tensor("out", [5, BL], f32, kind="ExternalOutput")
    dbg = {}
    if debug_dump:
        dbg["qkv"] = nc.dram_tensor("dbg_qkv", [96, CB, 25], f16, kind="ExternalOutput")
        dbg["att"] = nc.dram_tensor("dbg_att", [128, BP, 25], f16, kind="ExternalOutput")
        dbg["yt"] = nc.dram_tensor("dbg_yt", [64, CB, 25], f16, kind="ExternalOutput")
        dbg["y1"] = nc.dram_tensor("dbg_y1", [128, CB], f16, kind="ExternalOutput")

    in_names = [t.name for t in (x_d, wp32_d, wp16_d)]

    with tile.TileContext(nc) as tc, ExitStack() as ctx:
        singles = ctx.enter_context(tc.tile_pool(name="singles", bufs=1))
        imc_p = ctx.enter_context(tc.tile_pool(name="imc", bufs=2))
        qkv_p = ctx.enter_context(tc.tile_pool(name="qkv", bufs=2))
        yt_p = ctx.enter_context(tc.tile_pool(name="yt", bufs=3))
        qt_p = ctx.enter_context(tc.tile_pool(name="qt", bufs=3))
        att_p = ctx.enter_context(tc.tile_pool(name="att", bufs=2))
        z_p = ctx.enter_context(tc.tile_pool(name="z", bufs=3))
        e_p = ctx.enter_context(tc.tile_pool(name="e", bufs=3))
        red_p = ctx.enter_context(tc.tile_pool(name="red", bufs=3))
        mlp_p = ctx.enter_context(tc.tile_pool(name="mlp", bufs=2))
        outp_p = ctx.enter_context(tc.tile_pool(name="outp", bufs=2))
        ps_conv = ctx.enter_context(tc.tile_pool(name="psc", bufs=2, space="PSUM"))
        ps_att = ctx.enter_context(tc.tile_pool(name="psa", bufs=1, space="PSUM"))
        ps_mlp = ctx.enter_context(tc.tile_pool(name="psm", bufs=1, space="PSUM"))
        ps_red = ctx.enter_context(tc.tile_pool(name="psr", bufs=2, space="PSUM"))

        # ---- weight loads: small conv/attn weights first so chunk-0's x
        # load isn't stuck behind the 835KB MLP-weight transfer (that part
        # is issued inside the c==0 body, after the first x DMA) ----
        wp32 = singles.tile([128, 5], f32)
        wp16 = singles.tile([128, 4229], f16)
        # wconv (16KB) gates the first conv matmul: load it before anything
        # else; conv biases next; aw/w3/identity later; the 835KB MLP block
        # last (inside the c==0 body, behind the first x DMA).
        nc.sync.dma_start(out=wp16[0:32, 3973:4229],
                          in_=wp16_d.ap()[0:32, 3973:4229])
        nc.sync.dma_start(out=wp32, in_=wp32_d.ap())
        nc.sync.dma_start(out=wp16[:, 3264:3333],
                          in_=wp16_d.ap()[:, 3264:3333])
        nc.sync.dma_start(out=wp16[:, 3845:3973],
                          in_=wp16_d.ap()[:, 3845:3973])
        cb_s = wp32[:, 0:1]
        expb_s = wp32[:, 1:2]
        b1_s = wp32[:, 2:3]
        b2_s = wp32[0:64, 3:4]
        ab_s = wp32[:, 4:5]             # attn bias tiled 4x across partitions
        w2_s = wp16[:, 0:64]
        w1_s = wp16[0:64, 64:3264].rearrange("p (h i) -> p h i", i=25)
        aw_s = wp16[:, 3264:3328]       # block-diag aw per (b4-pair, b4%2)
        w3_s = wp16[0:64, 3328:3333]
        ident_s = wp16[:, 3845:3973]    # fp16 eye(128) for j-sum matmuls
        # K=32 im2col conv weights, one [32,128] set per dy
        wconv32_s = wp16[0:32, 3973:4229].rearrange("p (d c) -> p d c", d=2)

        NB = 512 // 25                              # 20 samples per col-tile
        NTILE = (CB + NB - 1) // NB                 # 7 col-tiles per chunk

        def conv_head(c):
            b0 = (c % NCHUNK) * CB
            # ---- x load as K=32 im2col over dx: xa32[(dx, cin), b, 35] ----
            # two dx-shifted contiguous-window DMAs (3-dim APs -- the DMA
            # layer caps at 3 dims); dy becomes a 2-step PSUM accumulation.
            xa32 = imc_p.tile([32, CB, 36], f16)
            for dx in range(2):
                src = bass.AP(tensor=x_d, offset=b0 * 576 + dx,
                              ap=[[36, 16], [576, CB], [1, 35]])
                nc.sync.dma_start(
                    out=xa32[16 * dx:16 * dx + 16, :, 0:35], in_=src)
            if c == 0:
                nc.sync.dma_start(out=wp16[:, 0:3264],
                                  in_=wp16_d.ap()[:, 0:3264])
            qkv_s = qkv_p.tile([96, CB, 25], f16)   # rows: q(0:32) k(32:64) v(64:96)
            y_t = yt_p.tile([64, CB, 25], f16)      # 0:32 relu(conv_out), 32:64 attn
            q_t = qt_p.tile([128, BP, 26], f16, tag="q_t")     # lane 25 pad
            k_t = qt_p.tile([128, BP, 25], f16, tag="k_t")
            k_dd = qt_p.tile([128, BP, 25, 2], f16, tag="k_dd")
            v_t = qt_p.tile([128, BP, 25], f16, tag="v_t")
            v_dd = qt_p.tile([128, BP, 25, 2], f16, tag="v_dd")
            return dict(xa32=xa32, qkv_s=qkv_s, y_t=y_t, q_t=q_t, k_t=k_t,
                        k_dd=k_dd, v_t=v_t, v_dd=v_dd)

        def conv_tiles(st, t_lo, t_hi):
            # ---- conv as 2 accumulating K=32 matmuls per col-tile;
            # emitted in pieces interleaved between reduction groups so
            # the PE/ACT streams never see a long conv-only block ----
            xa32, qkv_s, y_t = st["xa32"], st["qkv_s"], st["y_t"]
            for t in range(t_lo, min(t_hi, NTILE)):
                s0 = t * NB
                sb_n = min(NB, CB - s0)
                tn = sb_n * 25
                pscv = ps_conv.tile([128, 512], f32)
                for dy in range(2):
                    rhs = bass.AP(
                        tensor=xa32.tensor,
                        offset=xa32.offset + s0 * 36 + dy * 6,
                        ap=[xa32.ap[0], [36, sb_n], [6, 5], [1, 5]],
                    )
                    nc.tensor.matmul(pscv[:, :tn], wconv32_s[:, dy],
                                     rhs, start=(dy == 0), stop=(dy == 1))
                # qkv rows (0:96) first: the shuffle chain waits on these
                nc.scalar.activation(
                    qkv_s[:, s0:s0 + sb_n],
                    pscv[0:96, :tn].rearrange("p (b i) -> p b i", i=25),
                    AF.Identity, bias=cb_s[0:96], scale=1.0)
                nc.scalar.activation(
                    y_t[0:32, s0:s0 + sb_n],
                    pscv[96:128, :tn].rearrange("p (b i) -> p b i", i=25),
                    AF.Relu, bias=cb_s[96:128], scale=1.0)

        def conv_shuffles(st):
            # ---- shuffle qkv -> pair-major layout [ (b4, n), b', . ] ----
            # z/E/Ev free dims are (b', j, i) with i innermost, so the
            # per-j reduction matmul rhs slices are contiguous 50B runs.
            qkv_v = st["qkv_s"].rearrange("p (b2 f) i -> p f b2 i", f=4)
            # issue order q -> k -> v: SWDGE issue is ~1us serial per DMA,
            # and the z chain needs q_t + k_dd first, v_dd last
            for b4 in range(4):
                nc.gpsimd.dma_start(out=st["q_t"][32 * b4:32 * b4 + 32, :, 0:25],
                                    in_=qkv_v[0:32, b4])
            for b4 in range(4):
                nc.gpsimd.dma_start(out=st["k_t"][32 * b4:32 * b4 + 32],
                                    in_=qkv_v[32:64, b4])
            for b4 in range(4):
                nc.gpsimd.dma_start(out=st["v_t"][32 * b4:32 * b4 + 32],
                                    in_=qkv_v[64:96, b4])

        def dup_stage(st):
            # pair-duplication copies on DVE, emitted after the z/exp
            # prefetch of the nearer chunk: any stall waiting for the k/v
            # shuffles lands in DVE's boundary idle, never ahead of an exp
            # on the ACT queue.
            nc.vector.tensor_copy(st["k_dd"][:, :, :, 0:1],
                                  st["k_t"].unsqueeze(3))
            nc.vector.tensor_copy(st["k_dd"][:, :, :, 1:2],
                                  st["k_t"].unsqueeze(3))
            nc.vector.tensor_copy(st["v_dd"][:, :, :, 0:1],
                                  st["v_t"].unsqueeze(3))
            nc.vector.tensor_copy(st["v_dd"][:, :, :, 1:2],
                                  st["v_t"].unsqueeze(3))

        def emit_z(st, s):
            q_t, k_dd = st["q_t"], st["k_dd"]
            z = z_p.tile([128, SB, 25, 26], f16)
            q_bc = bass.AP(tensor=q_t.tensor, offset=q_t.offset + SB * s * 26,
                           ap=[q_t.ap[0], [26, SB], [0, 25], [1, 26]])
            k_bc = bass.AP(tensor=k_dd.tensor, offset=k_dd.offset + SB * s * 50,
                           ap=[k_dd.ap[0], [50, SB], [2, 25], [0, 13], [1, 2]])
            nc.vector.tensor_mul(z, q_bc, k_bc)
            return z

        def emit_exp(z):
            # T[:,0] = E = exp(z+bias); T[:,1] = Ev (written by the DVE mul).
            # The i=25 pad lane is never read by the reduction (it streams
            # i 0:25), so no pad memsets are needed anywhere.
            T = e_p.tile([128, 2, SB, 25, 26], f16)
            nc.scalar.activation(T[:, 0, :, :, 0:25], z[:, :, :, 0:25],
                                 AF.Exp, bias=expb_s, scale=1.0)
            return T

        def attn_subs(c, st, pre, stn2=None):
            """Attention subs 0..3 of chunk c.  pre = (z-tiles for subs 0/1,
            T tile with sub-0's exp already emitted) prefetched by the
            previous chunk's tail, or None for chunk 0.  stn2 = chunk c+2's
            conv state: its col-tiles are emitted two per sub (plus the
            shuffles after the last), so conv MMs/evicts spread across the
            window instead of forming a boundary block."""
            v_dd = st["v_dd"]
            attn_t = att_p.tile([128, BP, 25], f16, tag="attn_t")
            st["attn_t"] = attn_t
            if pre is None:
                zq = [emit_z(st, 0), emit_z(st, 1)]
                Ts = {}
            else:
                zq = list(pre["z"])
                Ts = {0: pre["T0"]}
            for s in range(NSUB):
                sl = slice(SB * s, SB * (s + 1))
                v_bc = bass.AP(tensor=v_dd.tensor, offset=v_dd.offset + SB * s * 50,
                               ap=[v_dd.ap[0], [50, SB], [2, 25], [0, 13], [1, 2]])
                T = Ts.pop(s, None)
                if T is None:
                    T = emit_exp(zq[s])
                if s + 2 < NSUB:
                    zq.append(emit_z(st, s + 2))
                nc.vector.tensor_mul(T[:, 1], T[:, 0], v_bc)
                # j-sum on PE: psr[:,0] = den = sum_j E, psr[:,1] = num.
                # One explicit identity ldweights per group; the 25
                # accumulating matmuls skip their per-matmul self-load.
                psr = ps_red.tile([128, 2, SB, 25], f32)
                psr2 = psr.rearrange("p a b i -> p (a b i)")
                nc.tensor.ldweights(ident_s)
                for j in range(25):
                    mm = nc.tensor.matmul(psr2, ident_s, T[:, :, :, j, 0:25],
                                          start=(j == 0), stop=(j == 24))
                    mm.ins.ldweights = False
                den, num = psr[:, 0], psr[:, 1]
                rden = red_p.tile([128, SB, 25, 1], f32, tag="rden")
                nc.vector.reciprocal_approx_fast(
                    rden.rearrange("p b i o -> p (b i o)"),
                    den.rearrange("p b i -> p (b i)"))
                nc.vector.tensor_mul(attn_t[:, sl, :].unsqueeze(3),
                                     num.unsqueeze(3), rden)
                if stn2 is not None:
                    conv_tiles(stn2, 2 * s, 2 * s + 2)
                    if s == NSUB - 1:
                        conv_tiles(stn2, 2 * NSUB, NTILE)
                        conv_shuffles(stn2)

        def prefetch_next(stn):
            """Cross-chunk prefetch: z for subs 0/1 and exp for sub 0 of the
            next chunk, so the next chunk's first reduction group has its
            inputs ready the moment the PE reaches it."""
            zn0 = emit_z(stn, 0)
            zn1 = emit_z(stn, 1)
            return {"z": [zn0, zn1], "T0": emit_exp(zn0)}

        def attn_tail(c, st):
            b0 = (c % NCHUNK) * CB
            y_t, attn_t = st["y_t"], st["attn_t"]
            # ---- 1x1 conv + relu -> y_t rows 32:64 ----
            # contract v-channels inside the b4 partition blocks of pair-major
            # attn_t directly (block-diag aw, two b4 per matmul at bases 0/64)
            # -> no attention re-shuffle DMAs.
            for g in range(2):
                atf = attn_t[64 * g:64 * g + 64].rearrange("p b i -> p (b i)")
                psat = ps_att.tile([64, BP * 25], f32)
                for t0 in range(0, BP * 25, 512):
                    tn = min(512, BP * 25 - t0)
                    nc.tensor.matmul(psat[:, t0:t0 + tn],
                                     aw_s[64 * g:64 * g + 64],
                                     atf[:, t0:t0 + tn], start=True, stop=True)
                for h in range(2):
                    b4 = 2 * g + h
                    # y_t columns for samples b = b4 + 4*b'
                    ysl = y_t[32:64]
                    ydst = bass.AP(tensor=ysl.tensor,
                                   offset=ysl.offset + b4 * 25,
                                   ap=[ysl.ap[0], [100, BP], [1, 25]])
                    nc.scalar.activation(
                        ydst,
                        psat[32 * h:32 * h + 32].rearrange(
                            "p (b i) -> p b i", i=25),
                        AF.Relu, bias=ab_s[64 * g + 32 * h:64 * g + 32 * h + 32],
                        scale=1.0)

            # ---- dense1: accumulate over 25 pixels ----
            ps1 = ps_mlp.tile([128, CB], f32, tag="ps1")
            for i in range(25):
                nc.tensor.matmul(ps1, w1_s[:, :, i], y_t[:, :, i],
                                 start=(i == 0), stop=(i == 24))
            y1 = mlp_p.tile([128, CB], f16, tag="y1")
            nc.scalar.activation(y1, ps1, AF.Relu, bias=b1_s, scale=1.0)

            # ---- dense2 (shares one PSUM bank with dense3) ----
            ps23 = ps_mlp.tile([64, 2, CB], f32, tag="ps23")
            nc.tensor.matmul(ps23[:, 0], w2_s, y1, start=True, stop=True)
            y2 = mlp_p.tile([64, CB], f16, tag="y2")
            nc.scalar.activation(y2, ps23[:, 0], AF.Relu, bias=b2_s, scale=1.0)

            if debug_dump and c == 0:
                nc.sync.dma_start(out=dbg["qkv"].ap(), in_=st["qkv_s"])
                nc.sync.dma_start(out=dbg["att"].ap(), in_=attn_t)
                nc.sync.dma_start(out=dbg["yt"].ap(), in_=y_t)
                nc.sync.dma_start(out=dbg["y1"].ap(), in_=y1)

            # ---- dense3 (bias added on host) ----
            nc.tensor.matmul(ps23[0:5, 1], w3_s, y2, start=True, stop=True)
            outs = outp_p.tile([5, CB], f32)
            nc.scalar.copy(outs, ps23[0:5, 1])
            nc.sync.dma_start(
                out=bass.AP(tensor=out_d, offset=b0, ap=[[BL, 5], [1, CB]]),
                in_=outs)

        # 2-deep software pipeline: conv for chunk c+2 is emitted inside
        # chunk c's window (after the attention subs), and z/exp for chunk
        # c+1's first subs are prefetched at chunk c's tail, so neither the
        # ACT eviction backlog nor the shuffle chain ever sits in front of
        # the exp -> Ev -> reduction critical path and the PE never idles
        # long enough for HAM to re-throttle.
        NTOT = NCHUNK * repeat

        def full_conv(c):
            st = conv_head(c)
            conv_tiles(st, 0, NTILE)
            conv_shuffles(st)
            return st

        sts = {0: full_conv(0)}
        dup_stage(sts[0])
        if NTOT > 1:
            sts[1] = full_conv(1)
            dup_stage(sts[1])
        pre = None
        for c in range(NTOT):
            stn2 = None
            if c + 2 < NTOT:
                stn2 = conv_head(c + 2)
                sts[c + 2] = stn2
            attn_subs(c, sts[c], pre, stn2)
            pre = prefetch_next(sts[c + 1]) if c + 1 < NTOT else None
            if stn2 is not None:
                dup_stage(stn2)
            attn_tail(c, sts[c])
            del sts[c]

    nc.finalize()
    return nc, in_names, out_d.name


_PROG = None


def _get_program():
    global _PROG
    if _PROG is None:
        _PROG = _build_program()
    return _PROG


def _host_conv(x, w, b):
    """2x2 VALID conv, NCHW, numpy. Returns [B, O, 25] float32."""
    B_, C_, H_, W_ = x.shape
    out = None
    for dy in range(2):
        for dx in range(2):
            xs = x[:, :, dy:dy + 5, dx:dx + 5].reshape(B_, C_, 25)
            t = np.einsum('oc,bcp->bop', w[:, :, dy, dx], xs,
                          optimize=True)
            out = t if out is None else out + t
    return (out + b[None, :, None]).astype(np.float32)


def _make_in_maps(inputs):
    return _host_prep(**inputs)


def _host_prep(x, conv_w, conv_b, qkv_w, qkv_b, attn_w, attn_b,
               w1, b1, w2, b2, w3, b3):
    # host-side weight prep
    # channel order [q|k|v|conv_out]
    wc = np.concatenate([np.asarray(qkv_w), np.asarray(conv_w)], axis=0)  # [128,16,2,2]
    # wconv32[(dx, cin), dy, ch] for the 2-step K=32 im2col conv matmuls
    wconv32 = np.ascontiguousarray(
        wc.transpose(3, 1, 2, 0).reshape(32, 2, 128)).astype(np.float16)
    cbias = np.concatenate([np.asarray(qkv_b), np.asarray(conv_b)])[:, None].astype(np.float32)
    aw = np.ascontiguousarray(np.asarray(attn_w)[:, :, 0, 0].T).astype(np.float16)
    ab = np.asarray(attn_b)[:, None].astype(np.float32)
    w1t = np.ascontiguousarray(
        np.asarray(w1).reshape(HID, 64, 25).transpose(1, 0, 2)).astype(np.float16)
    b1c = np.asarray(b1)[:, None].astype(np.float32)
    w2t = np.ascontiguousarray(np.asarray(w2).T).astype(np.float16)
    b2c = np.asarray(b2)[:, None].astype(np.float32)
    w3t = np.ascontiguousarray(np.asarray(w3).T).astype(np.float16)

    x = np.asarray(x, dtype=np.float32)

    # exp-overflow guard: softmax is invariant to exp(z - C0); pick C0 from the
    # exact global max of q_i*k_j (corner products of per-row min/max).
    qw, kw = np.asarray(qkv_w)[0:32], np.asarray(qkv_w)[32:64]
    qb_, kb_ = np.asarray(qkv_b)[0:32], np.asarray(qkv_b)[32:64]
    qv = _host_conv(x, qw, qb_)        # [B, 32, 25]
    kv = _host_conv(x, kw, kb_)
    qmax, qmin = qv.max(2), qv.min(2)  # [B, 32]
    kmax, kmin = kv.max(2), kv.min(2)
    zmax = max((qmax * kmax).max(), (qmax * kmin).max(),
               (qmin * kmax).max(), (qmin * kmin).max())
    # margin 5.5: keeps E = exp(z-c0) and Ev comfortably inside fp16 range
    # (num/den partial sums now accumulate in fp32 PSUM).
    c0 = float(max(0.0, zmax - 5.5))
    expb = np.full((128, 1), -c0, dtype=np.float32)

    wp32 = np.zeros((128, 5), np.float32)
    wp32[:, 0:1] = cbias
    wp32[:, 1:2] = expb
    wp32[:, 2:3] = b1c
    wp32[0:64, 3:4] = b2c
    wp32[:, 4:5] = np.tile(ab, (4, 1))
    wp16 = np.zeros((128, 4229), np.float16)
    wp16[:, 0:64] = w2t
    wp16[0:64, 64:3264] = w1t.reshape(64, 3200)
    # block-diag aw: row-block b4 gets aw at col-block (b4 % 2)
    for b4 in range(4):
        wp16[32 * b4:32 * b4 + 32,
             3264 + 32 * (b4 % 2):3296 + 32 * (b4 % 2)] = aw
    wp16[0:64, 3328:3333] = w3t
    wp16[:, 3845:3973] = np.eye(128, dtype=np.float16)
    wp16[0:32, 3973:4229] = wconv32.reshape(32, 256)

    x16 = x.astype(np.float16)
    shared = {"wp32": wp32, "wp16": wp16}
    in_maps = []
    for c in range(NCORES):
        m = dict(shared)
        m["x_s"] = np.ascontiguousarray(x16[c * BL:(c + 1) * BL])
        in_maps.append(m)
    return in_maps


def kernel(x, conv_w, conv_b, qkv_w, qkv_b, attn_w, attn_b,
           w1, b1, w2, b2, w3, b3):
    from concourse.bass_utils import run_bass_kernel_spmd

    nc, in_names, out_name = _get_program()
    in_maps = _host_prep(x, conv_w, conv_b, qkv_w, qkv_b, attn_w, attn_b,
                         w1, b1, w2, b2, w3, b3)
    res = run_bass_kernel_spmd(nc, in_maps, core_ids=list(range(NCORES)))
    outs = [r[out_name] for r in res.results]           # each [5, BL]
    full = np.concatenate([o.T for o in outs], axis=0)  # [8192, 5]
    full = full + np.asarray(b3)[None, :].astype(np.float32)
    return full.astype(np.float32)

